# revision 8
# baseline (speedup 1.0000x reference)
"""Distributed CLIP-style loss (l2i symmetric CE + g2i NT-Xent) on 8 TRN2 cores.

Strategy: data-parallel row sharding. Each core k receives column-ROTATED
transposed feature matrices (rotation = its global row offset), so the
diagonal (pos-pair) blocks sit at static local column offsets and one SPMD
program serves all 8 cores. Each core computes the LSE rows for its 256
image rows, 256 text rows and 512 z rows; the host sums the per-row partials.

All GEMMs run in bf16 on the PE (fp32 PSUM accumulate). z-normalization is
computed on-device: square -> ones-matmul (partition-broadcast sum(z^2)) ->
reciprocal -> sqrt -> bf16 scale multiply.
"""

import numpy as np
import ml_dtypes

import concourse.bass as bass
import concourse.mybir as mybir
from concourse.tile import TileContext
from concourse.vector_clock import ScopedClock
from concourse import bass_utils


# --- compat patches for the walrus build in this container ---------------
# 1) EVENT_SEMAPHORE_RANGE_CLEAR (InstISA op 176) is rejected ("ISA wrong
#    length"); emit one EventSemaphore sem-wr-imm 0 per semaphore instead.
def _sem_clear_compat(self, sem):
    nums = list(sem) if isinstance(sem, range) else [
        sem.num if hasattr(sem, "num") else int(sem)
    ]
    last = None
    for n in nums:
        last = self.add_instruction(
            mybir.InstEventSemaphore(
                name=self.bass.get_next_instruction_name(),
                ins=[], outs=[],
                sync_info=mybir.SyncInfo(
                    on_wait=[],
                    on_update=[mybir.SyncUpdate(
                        sync_type="semaphore", id=n,
                        update_mode="sem-wr-imm", update_value=0)],
                ),
            )
        )
    return last


bass.BassGpSimd.sem_clear = _sem_clear_compat


# 2) Every instruction in this walrus build has a single sync-wait slot
#    ("Too many sync wait commands" otherwise), while Tile freely attaches
#    several. Post-pass: hoist extra waits onto wait-only EventSemaphore
#    instructions inserted immediately before the instruction on the same
#    engine (sequencers execute in order, so the semantics are identical).
_mw_ctr = [0]


def _split_multi_waits(nc: bass.Bass) -> None:
    for f in nc.m.functions:
        for bb in f.blocks:
            out = []
            changed = False
            for inst in bb.instructions:
                si = inst.sync_info
                waits = list(si.on_wait) if si is not None and si.on_wait else []
                if len(waits) > 1:
                    for w in waits[:-1]:
                        _mw_ctr[0] += 1
                        es = mybir.InstEventSemaphore(
                            name=f"I-mwsplit-{_mw_ctr[0]}",
                            engine=inst.engine,
                            ins=[], outs=[],
                            sync_info=mybir.SyncInfo(on_wait=[w], on_update=[]),
                        )
                        out.append(es)
                    inst.sync_info = mybir.SyncInfo(
                        on_wait=[waits[-1]],
                        on_update=list(si.on_update or []),
                    )
                    changed = True
                out.append(inst)
            if changed:
                bb.instructions = out
# -------------------------------------------------------------------------

B = 2048
D = 1024
NCORES = 8
TEMP = 0.05
INV_TEMP = 1.0 / TEMP
BPC = B // NCORES          # 256 image/text rows per core
ZPC = 2 * B // NCORES      # 512 z rows per core
NCH = D // 128             # 8 contraction chunks
NB_L = B // 512            # 4 psum banks per l2i row-tile
NB_G = 2 * B // 512        # 8 psum banks per g2i row-tile
NT_L = BPC // 128          # 2 l2i row-tiles per core
NT_G = ZPC // 128          # 4 g2i row-tiles per core

BF16 = mybir.dt.bfloat16
F32 = mybir.dt.float32
AF = mybir.ActivationFunctionType

# stats_out column layout ([128, 16] f32 per core)
COL_LSE_IMG = 0   # +t (2)
COL_LSE_TXT = 2   # +t (2)
COL_POS_L2I = 4   # +t (2) raw dot (unscaled)
COL_LSE_G2I = 6   # +t (4)
COL_POS_G2I = 10  # +t (4) raw cosine sim (unscaled)

_cache: dict = {}


def _build_program(ls: float) -> bass.Bass:
    nc = bass.Bass(trn_type="TRN2")
    img_d = nc.dram_tensor("img", [D, B], BF16, kind="ExternalInput")
    txt_d = nc.dram_tensor("txt", [D, B], BF16, kind="ExternalInput")
    z_d = nc.dram_tensor("z", [D, 2 * B], BF16, kind="ExternalInput")
    eye_d = nc.dram_tensor("eye", [128, 128], F32, kind="ExternalInput")
    negeye_d = nc.dram_tensor("negeye", [128, 128], F32, kind="ExternalInput")
    ones_d = nc.dram_tensor("ones", [128, 128], BF16, kind="ExternalInput")
    out_d = nc.dram_tensor("out", [128, 16], F32, kind="ExternalOutput")

    with TileContext(nc) as tc:
        with (
            tc.tile_pool(name="consts", bufs=1) as consts,
            tc.tile_pool(name="feat", bufs=8) as featp,
            tc.tile_pool(name="zstream", bufs=3) as zp,
            tc.tile_pool(name="zsq", bufs=8) as zsqp,
            tc.tile_pool(name="norm", bufs=1) as normp,
            tc.tile_pool(name="stats", bufs=10) as statp,
            tc.tile_pool(name="scratch", bufs=4) as scrp,
            tc.tile_pool(name="mm", bufs=8, space="PSUM") as mmp,
        ):
            eye = consts.tile([128, 128], F32, tag="eye")
            negeye = consts.tile([128, 128], F32, tag="negeye")
            ones = consts.tile([128, 128], BF16, tag="ones")
            nc.sync.dma_start(eye, eye_d[:, :])
            nc.sync.dma_start(negeye, negeye_d[:, :])
            nc.sync.dma_start(ones, ones_d[:, :])

            stats_out = consts.tile([128, 16], F32, tag="statsout")
            nc.vector.memset(stats_out, 0.0)

            # ---- input DMAs: img/txt chunk-interleaved, then z (pass 1) ----
            img_c = []
            txt_c = []
            for c in range(NCH):
                it = featp.tile([128, B], BF16, tag="img")
                tt = featp.tile([128, B], BF16, tag="txt")
                nc.sync.dma_start(it, img_d[c * 128:(c + 1) * 128, :])
                nc.sync.dma_start(tt, txt_d[c * 128:(c + 1) * 128, :])
                img_c.append(it)
                txt_c.append(tt)

            # ================= Phase A: l2i (two sides) =================
            for side in range(2):
                lhs_c = img_c if side == 0 else txt_c
                rhs_c = txt_c if side == 0 else img_c
                ps = [[None] * NB_L for _ in range(NT_L)]
                for t in range(NT_L):
                    for b in range(NB_L):
                        ps[t][b] = mmp.tile([128, 512], F32, tag="ps", name="ps")
                for c in range(NCH):
                    for t in range(NT_L):
                        for b in range(NB_L):
                            nc.tensor.matmul(
                                ps[t][b],
                                lhs_c[c][:, t * 128:(t + 1) * 128],
                                rhs_c[c][:, b * 512:(b + 1) * 512],
                                start=(c == 0),
                                stop=(c == NCH - 1),
                            )
                for t in range(NT_L):
                    maxs = statp.tile([128, NB_L], F32, tag="maxs")
                    sums = statp.tile([128, NB_L], F32, tag="sums")
                    negmax = statp.tile([128, NB_L], F32, tag="negmax")
                    for b in range(NB_L):
                        if side == 0 and b == 0:
                            # raw positive dot: diag of the [128,128] block
                            scr = scrp.tile([128, 128], F32, tag="ttrscr")
                            nc.vector.tensor_mul(
                                scr, ps[t][0][:, t * 128:(t + 1) * 128], eye)
                            nc.vector.reduce_sum(
                                stats_out[:, COL_POS_L2I + t:COL_POS_L2I + t + 1],
                                scr, axis=mybir.AxisListType.X)
                        nc.vector.reduce_max(
                            maxs[:, b:b + 1], ps[t][b], axis=mybir.AxisListType.X
                        )
                        nc.scalar.mul(negmax[:, b:b + 1], maxs[:, b:b + 1], -ls)
                        escr = scrp.tile([128, 512], BF16, tag="escr")
                        nc.scalar.activation(
                            escr, ps[t][b], AF.Exp,
                            bias=negmax[:, b:b + 1], scale=ls,
                            accum_out=sums[:, b:b + 1],
                        )
                    # combine banks: lse = ls*gmax + ln(sum_b S_b * exp(ls*(max_b-gmax)))
                    gmax = statp.tile([128, 1], F32, tag="gmax")
                    nc.vector.reduce_max(gmax, maxs, axis=mybir.AxisListType.X)
                    neggmax = statp.tile([128, 1], F32, tag="neggmax")
                    nc.scalar.mul(neggmax, gmax, -ls)
                    w4 = statp.tile([128, NB_L], F32, tag="w4")
                    nc.scalar.activation(w4, maxs, AF.Exp, bias=neggmax, scale=ls)
                    scr4 = statp.tile([128, NB_L], F32, tag="scr4")
                    S = statp.tile([128, 1], F32, tag="S")
                    nc.vector.tensor_mul(scr4, sums, w4)
                    nc.vector.reduce_sum(S, scr4, axis=mybir.AxisListType.X)
                    lnS = statp.tile([128, 1], F32, tag="lnS")
                    nc.scalar.activation(lnS, S, AF.Ln)
                    gms = statp.tile([128, 1], F32, tag="gms")
                    nc.scalar.mul(gms, gmax, ls)
                    col = (COL_LSE_IMG if side == 0 else COL_LSE_TXT) + t
                    nc.vector.tensor_add(stats_out[:, col:col + 1], lnS, gms)

            # ================= Phase B: z norms =================
            zsq_c = []
            for c in range(NCH):
                zt = zp.tile([128, 2 * B], BF16, tag="zstream")
                nc.sync.dma_start(zt, z_d[c * 128:(c + 1) * 128, :])
                sq = zsqp.tile([128, 2 * B], BF16, tag="zsq")
                nc.scalar.activation(sq, zt, AF.Square)
                zsq_c.append(sq)

            # nsq[j] broadcast to all partitions via ones-matmul, c-outer
            nps = [mmp.tile([128, 512], F32, tag="ps", name="ps") for _ in range(NB_G)]
            for c in range(NCH):
                for b in range(NB_G):
                    nc.tensor.matmul(
                        nps[b], ones, zsq_c[c][:, b * 512:(b + 1) * 512],
                        start=(c == 0), stop=(c == NCH - 1),
                    )
            recip = normp.tile([128, 2 * B], F32, tag="recip")
            invb = normp.tile([128, 2 * B], BF16, tag="invb")
            for b in range(NB_G):
                sl = slice(b * 512, (b + 1) * 512)
                nc.vector.reciprocal(recip[:, sl], nps[b])
                nc.scalar.activation(invb[:, sl], recip[:, sl], AF.Sqrt)

            # pass 2: zn = z * invnorm (bf16, DVE 2x mode)
            zn_c = []
            for c in range(NCH):
                zt = zp.tile([128, 2 * B], BF16, tag="zstream")
                nc.sync.dma_start(zt, z_d[c * 128:(c + 1) * 128, :])
                zn = zsqp.tile([128, 2 * B], BF16, tag="zsq")
                nc.vector.tensor_mul(zn, zt, invb)
                zn_c.append(zn)

            # ================= Phase C: g2i =================
            for t in range(NT_G):
                ps = [mmp.tile([128, 512], F32, tag="ps", name="ps") for _ in range(NB_G)]
                if t == 0:
                    for c in range(NCH):
                        for b in range(NB_G):
                            nc.tensor.matmul(
                                ps[b],
                                zn_c[c][:, t * 128:(t + 1) * 128],
                                zn_c[c][:, b * 512:(b + 1) * 512],
                                start=(c == 0), stop=(c == NCH - 1),
                            )
                else:
                    for b in range(NB_G):
                        for c in range(NCH):
                            nc.tensor.matmul(
                                ps[b],
                                zn_c[c][:, t * 128:(t + 1) * 128],
                                zn_c[c][:, b * 512:(b + 1) * 512],
                                start=(c == 0), stop=(c == NCH - 1),
                            )
                sums8 = statp.tile([128, NB_G], F32, tag="sums8")
                for b in range(NB_G):
                    if b == 0:
                        # mask self-similarity diagonal with -1e30
                        blk = ps[0][:, t * 128:(t + 1) * 128]
                        nc.vector.tensor_add(blk, blk, negeye)
                    if b == NB_G // 2:
                        # positive pair: col (row + 2048) -> bank 4 diag block
                        scr = scrp.tile([128, 128], F32, tag="ttrscr")
                        nc.vector.tensor_mul(
                            scr, ps[b][:, t * 128:(t + 1) * 128], eye)
                        nc.vector.reduce_sum(
                            stats_out[:, COL_POS_G2I + t:COL_POS_G2I + t + 1],
                            scr, axis=mybir.AxisListType.X)
                    escr = scrp.tile([128, 512], BF16, tag="escr")
                    nc.scalar.activation(
                        escr, ps[b], AF.Exp, scale=INV_TEMP,
                        accum_out=sums8[:, b:b + 1],
                    )
                Ssum = statp.tile([128, 1], F32, tag="Ssum")
                nc.vector.reduce_sum(Ssum, sums8, axis=mybir.AxisListType.X)
                nc.scalar.activation(
                    stats_out[:, COL_LSE_G2I + t:COL_LSE_G2I + t + 1], Ssum, AF.Ln
                )

            nc.sync.dma_start(out_d[:, :], stats_out)

    _split_multi_waits(nc)
    return nc


def _get_program(ls: float) -> bass.Bass:
    key = float(ls)
    if key not in _cache:
        _cache[key] = _build_program(key)
    return _cache[key]


def kernel(image_features, gli_features, text_features, logit_scale):
    ls = float(np.asarray(logit_scale))
    nc = _get_program(ls)

    bf = ml_dtypes.bfloat16
    imgT = np.ascontiguousarray(np.asarray(image_features, dtype=np.float32).T)
    txtT = np.ascontiguousarray(np.asarray(text_features, dtype=np.float32).T)
    z = np.concatenate(
        [np.asarray(gli_features, dtype=np.float32),
         np.asarray(image_features, dtype=np.float32)], axis=0)
    zT = np.ascontiguousarray(z.T)

    eye = np.eye(128, dtype=np.float32)
    negeye = (-1e30 * np.eye(128)).astype(np.float32)
    ones = np.ones((128, 128), dtype=bf)

    in_maps = []
    for k in range(NCORES):
        in_maps.append({
            "img": np.ascontiguousarray(np.roll(imgT, -BPC * k, axis=1)).astype(bf),
            "txt": np.ascontiguousarray(np.roll(txtT, -BPC * k, axis=1)).astype(bf),
            "z": np.ascontiguousarray(np.roll(zT, -ZPC * k, axis=1)).astype(bf),
            "eye": eye,
            "negeye": negeye,
            "ones": ones,
        })

    res = bass_utils.run_bass_kernel_spmd(nc, in_maps, core_ids=list(range(NCORES)))
    globals()["LAST_RESULT"] = res
    out = np.stack([r["out"] for r in res.results]).astype(np.float64)  # [8,128,16]

    lse_img = out[:, :, COL_LSE_IMG:COL_LSE_IMG + NT_L].sum()
    lse_txt = out[:, :, COL_LSE_TXT:COL_LSE_TXT + NT_L].sum()
    pos_l2i = out[:, :, COL_POS_L2I:COL_POS_L2I + NT_L].sum()
    l2i = 0.5 * ((lse_img - ls * pos_l2i) / B + (lse_txt - ls * pos_l2i) / B)

    lse_g2i = out[:, :, COL_LSE_G2I:COL_LSE_G2I + NT_G].sum()
    pos_g2i = out[:, :, COL_POS_G2I:COL_POS_G2I + NT_G].sum()
    g2i = (lse_g2i - INV_TEMP * pos_g2i) / (2 * B)

    total = l2i + g2i
    return (np.float32(total), np.float32(l2i), np.float32(g2i))
